# revision 3
# baseline (speedup 1.0000x reference)
# Trainium2 Bass kernel v3 for nn_Attention_48052094107920 (sparse_attention).
#
# Sharding: core c = (head-group c//2, batch-half c%2). Each core processes
# 4 batches x 3 local heads x full query range. The mask-mix maps
# mw[g,h] = sum_m mask_proj[m, g*12+h] * masks_m are folded on the host
# (batch-independent weight fusion) and enter as fp16 inputs.
#
# On-chip layout: "k-major" score tiles S^T[k, q] (k on partitions); x^T is
# zero-padded to [768, 640] (no ones-row; the softmax-denominator column of
# v-hat is memset to 1 on chip). exp uses a -30 bias on padded k rows.
# Softmax: Z (ones-column of p@v PSUM) is copied to SBUF with the o rows,
# broadcast across partitions by DMA, and o/Z runs as an f32 divide (avoids
# fp16-subnormal 1/Z). Out-projection partials stage through fp16 SBUF.
#
# PSUM (8 banks): ppA [128,512]x2 (projections + out-proj), psS [128,4,256]x2
# (score rows j0-3), psT [128,256]x1 (score row j4 / pv tail), pov [65,512]x1.
#
# Engine split: PE matmuls; DVE most mask-mix + divides; Pool (gpsimd) rest of
# mask-mix + most psum->sbuf copies + memsets; ACT exp + wide score copies.

import numpy as np

import concourse.bass as bass
import concourse.bacc as bacc_mod
import concourse.mybir as mybir
import concourse.tile as tile
from concourse import bass_utils

BF = mybir.dt.float16
F32 = mybir.dt.float32
AF = mybir.ActivationFunctionType
OP = mybir.AluOpType

B, N, C = 8, 577, 768
GH, LH, ML, HD = 3, 12, 3, 64
NH = 3            # local heads per core
BC = 4            # batches per core
SCALE = HD ** -0.5
NP, NJ = 640, 5   # padded k tokens, k chunks of 128
NQ = 584          # padded q tokens
KO = 6            # contraction chunks (768 = 6*128)
VW = HD + 1       # 65 cols per head in v-hat: [v | ones]
EXP_NEG = -30.0
QCH = ((0, 256), (256, 512), (512, NQ))        # q chunks (scores / pv)
OQCH = ((0, 128), (128, 256), (256, 384), (384, 512), (512, NQ))  # out rows


def build_nc3(linearize=False):
    nc = bacc_mod.Bacc("TRN2", target_bir_lowering=False, debug=False, num_devices=8)

    xta = nc.dram_tensor("xta", [BC, 128, KO, NP], BF, kind="ExternalInput")
    wq = nc.dram_tensor("wq", [128, KO, GH * HD], BF, kind="ExternalInput")
    wk = nc.dram_tensor("wk", [128, KO, GH * HD], BF, kind="ExternalInput")
    wv = nc.dram_tensor("wv", [128, KO, NH * VW], BF, kind="ExternalInput")
    pw2 = nc.dram_tensor("pw2", [128, C], BF, kind="ExternalInput")
    pw1 = nc.dram_tensor("pw1", [64, C], BF, kind="ExternalInput")
    mw = nc.dram_tensor("mw", [128, NH, GH, NJ, NQ], BF, kind="ExternalInput")
    out = nc.dram_tensor("op", [BC, NQ, C], BF, kind="ExternalOutput")

    with tile.TileContext(nc, linearize=linearize) as tc, \
         tc.tile_pool(name="const", bufs=1) as cpool, \
         tc.tile_pool(name="xb", bufs=2) as xpool, \
         tc.tile_pool(name="work", bufs=2) as wpool, \
         tc.tile_pool(name="amix", bufs=2) as mpool, \
         tc.tile_pool(name="attn", bufs=3) as apool, \
         tc.tile_pool(name="onp", bufs=2) as opool, \
         tc.tile_pool(name="zrp", bufs=2) as zpool, \
         tc.tile_pool(name="psA", bufs=2, space="PSUM") as ppA, \
         tc.tile_pool(name="psS", bufs=2, space="PSUM") as ppS, \
         tc.tile_pool(name="psT", bufs=1, space="PSUM") as ppT, \
         tc.tile_pool(name="psV", bufs=1, space="PSUM") as ppV:

        # x[0] ahead of every other transfer so batch 0 starts ASAP
        xb0 = xpool.tile([128, KO, NP], BF, tag="xb")
        nc.sync.dma_start(xb0[:], xta.ap()[0])

        wq_s = cpool.tile([128, KO, GH * HD], BF, tag="wq")
        nc.sync.dma_start(wq_s[:], wq.ap())
        wk_s = cpool.tile([128, KO, GH * HD], BF, tag="wk")
        nc.sync.dma_start(wk_s[:], wk.ap())
        wv_s = cpool.tile([128, KO, NH * VW], BF, tag="wv")
        nc.sync.dma_start(wv_s[:], wv.ap())
        pw2_s = cpool.tile([128, C], BF, tag="pw2")
        nc.sync.dma_start(pw2_s[:], pw2.ap())
        pw1_s = cpool.tile([64, C], BF, tag="pw1")
        nc.sync.dma_start(pw1_s[:], pw1.ap())
        mw_s = cpool.tile([128, NH, GH, NJ, NQ], BF, tag="mw")

        def load_mw(hh):
            # per-(head, g) chunks, emitted late so xb transfers slip between
            for g in range(GH):
                nc.sync.dma_start(mw_s[:, hh, g], mw.ap()[:, hh, g])

        def phase_a(b, xb=None):
            if xb is None:
                xb = xpool.tile([128, KO, NP], BF, tag="xb")
                nc.sync.dma_start(xb[:], xta.ap()[b])

            q01 = wpool.tile([128, NQ], BF, tag="q01")
            q2 = wpool.tile([64, NQ], BF, tag="q2")
            k01 = wpool.tile([128, NP], BF, tag="k01")
            k2 = wpool.tile([64, NP], BF, tag="k2")
            vtb = wpool.tile([128, NJ, NH * VW], BF, tag="vtb")

            # q projection: [192 rows, NQ]; (512, 72) free chunks
            for msl, mp, dst in ((slice(0, 128), 128, q01), (slice(128, 192), 64, q2)):
                for n0, n1 in ((0, 512), (512, NQ)):
                    ps = ppA.tile([128, 512], F32, tag="bigA", name="psA")[:mp, : n1 - n0]
                    for o in range(KO):
                        nc.tensor.matmul(ps, wq_s[:, o, msl], xb[:, o, n0:n1],
                                         start=(o == 0), stop=(o == KO - 1))
                    nc.scalar.copy(dst[:mp, n0:n1], ps)

            # k projection: [192 rows, NP]; (512, 128) free chunks
            for msl, mp, dst in ((slice(0, 128), 128, k01), (slice(128, 192), 64, k2)):
                for n0, n1 in ((0, 512), (512, NP)):
                    ps = ppA.tile([128, 512], F32, tag="bigA", name="psA")[:mp, : n1 - n0]
                    for o in range(KO):
                        nc.tensor.matmul(ps, wk_s[:, o, msl], xb[:, o, n0:n1],
                                         start=(o == 0), stop=(o == KO - 1))
                    nc.vector.tensor_copy(dst[:mp, n0:n1], ps)

            # v-hat projection: rows k, cols [v_h | 0] per head; ones col memset
            for kc in range(NJ):
                ps = ppA.tile([128, 512], F32, tag="bigA", name="psA")[:, : NH * VW]
                for o in range(KO):
                    nc.tensor.matmul(ps, xb[:, o, kc * 128:(kc + 1) * 128], wv_s[:, o, :],
                                     start=(o == 0), stop=(o == KO - 1))
                nc.scalar.copy(vtb[:, kc, :], ps)
            # ones column only on VALID k rows: padded k (j=4, p>=65) stays 0,
            # so pad keys contribute nothing to o or Z and exp needs no bias
            nc.gpsimd.memset(vtb[:, 0:4, HD::VW], 1.0)
            nc.gpsimd.memset(vtb[0:65, 4, HD::VW], 1.0)
            return q01, q2, k01, k2, vtb

        def phase_b_scores(b, q01, q2, k01, k2, vtb):
            def qg(g):
                return (q01[0:64], q01[64:128], q2[0:64])[g]

            def kg(g):
                return (k01[0:64], k01[64:128], k2[0:64])[g]

            # scores S^T[k, q] per global head -> fp16 ssb
            ssb = wpool.tile([128, NJ, GH, NQ], BF, tag="ssb")
            for g in range(GH):
                for qi, (q0, q1) in enumerate(QCH):
                    qw = q1 - q0
                    ps = ppS.tile([128, 4, 256], F32, tag="sc", name="psS")
                    pt = ppT.tile([128, 256], F32, tag="st", name="psT")
                    for j in range(4):
                        nc.tensor.matmul(ps[:, j, :qw], kg(g)[:, j * 128:(j + 1) * 128],
                                         qg(g)[:, q0:q1], start=True, stop=True)
                    nc.tensor.matmul(pt[:, :qw], kg(g)[:, 512:640],
                                     qg(g)[:, q0:q1], start=True, stop=True)
                    nc.scalar.copy(ssb[:, 0:4, g, q0:q1], ps[:, :, :qw])
                    nc.vector.tensor_copy(ssb[:, 4, g, q0:q1], pt[:, :qw])
            return ssb

        def phase_b_rest(b, ssb, vtb):
            on2 = opool.tile([128, NQ], BF, tag="on2")
            on3 = opool.tile([64, NQ], BF, tag="on3")
            ont = opool.tile([64, NQ], BF, tag="ont")

            for hh in range(NH):
                # mask-mix: at = sum_g ssb_g * mw[hh, g]   (DVE/Pool split,
                # j-halved so exp and p@v can start before the full chain ends)
                at = mpool.tile([128, NJ, NQ], BF, tag="at")
                tt = mpool.tile([128, NJ, NQ], BF, tag="tt")
                t2 = mpool.tile([128, NJ, NQ], BF, tag="t2")
                e = apool.tile([128, NJ, NQ], BF, tag="e")
                for hi, jh in enumerate((slice(0, 3), slice(3, NJ))):
                    tmul = nc.vector.tensor_mul if hi == 0 else nc.gpsimd.tensor_mul
                    nc.vector.tensor_mul(at[:, jh], ssb[:, jh, 0], mw_s[:, hh, 0, jh])
                    nc.gpsimd.tensor_mul(tt[:, jh], ssb[:, jh, 1], mw_s[:, hh, 1, jh])
                    tmul(t2[:, jh], ssb[:, jh, 2], mw_s[:, hh, 2, jh])
                    nc.vector.tensor_add(at[:, jh], at[:, jh], tt[:, jh])
                    nc.gpsimd.tensor_add(at[:, jh], at[:, jh], t2[:, jh])
                    nc.scalar.activation(e[:, jh], at[:, jh], AF.Exp)

                # p@v; Z lands in row 64 (early-j MMs overlap the j3:5 exp)
                pov = ppV.tile([VW, 512], F32, tag="ov", name="psV")
                pvt = ppT.tile([128, 256], F32, tag="st", name="psT")
                for qi, (q0, q1) in enumerate(QCH):
                    dstp = pov[:, q0:q1] if qi < 2 else pvt[:VW, : q1 - q0]
                    for j in range(NJ):
                        nc.tensor.matmul(dstp, vtb[:, j, hh * VW:(hh + 1) * VW],
                                         e[:, j, q0:q1], start=(j == 0), stop=(j == NJ - 1))
                zrc = zpool.tile([1, NQ], F32, tag="zrc")
                nc.vector.reciprocal(zrc[0:1, 0:512], pov[64:65, :])
                nc.vector.reciprocal(zrc[0:1, 512:NQ], pvt[64:65, : NQ - 512])
                zrt = zpool.tile([64, NQ], F32, tag="zr")
                nc.gpsimd.partition_broadcast(zrt[:], zrc[0:1, :])
                dst = (on2[0:64], ont[0:64], on3[0:64])[hh]
                nc.vector.tensor_mul(dst[:, 0:512], pov[0:64], zrt[:, 0:512])
                nc.vector.tensor_mul(dst[:, 512:NQ], pvt[0:64, : NQ - 512],
                                     zrt[:, 512:NQ])
                if hh == 1:
                    nc.sync.dma_start(on2[64:128, :], ont[:, :])

            # output projection partials -> fp16 staging -> DRAM
            for q0, q1 in OQCH:
                ow = q1 - q0
                outsb = opool.tile([128, C], BF, tag="outsb")
                for i, (n0, n1) in enumerate(((0, 512), (512, C))):
                    ps = ppA.tile([128, 512], F32, tag="bigA", name="psA")[:ow, : n1 - n0]
                    nc.tensor.matmul(ps, on2[:, q0:q1], pw2_s[:, n0:n1],
                                     start=True, stop=False)
                    nc.tensor.matmul(ps, on3[:, q0:q1], pw1_s[:, n0:n1],
                                     start=False, stop=True)
                    nc.scalar.copy(outsb[:ow, n0:n1], ps)
                nc.sync.dma_start(out.ap()[b, q0:q1, :], outsb[:ow, :])

        # software pipeline: batch b's scores go to the PE first, then batch
        # b+1's projections (they fill the PE while b's mix/exp/pv chain runs
        # on DVE/Pool/ACT), then the rest of batch b. mw chunks are emitted
        # late so the xb transfers stay ahead of them in the SP queue.
        prev = phase_a(0, xb=xb0)
        cur_s = phase_b_scores(0, *prev)
        cur_vtb = prev[4]
        for b in range(BC):
            nxt = phase_a(b + 1) if b + 1 < BC else None
            if b == 0:
                for hh in range(NH):
                    load_mw(hh)
            nxt_s = phase_b_scores(b + 1, *nxt) if nxt else None
            phase_b_rest(b, cur_s, cur_vtb)
            cur_s = nxt_s
            cur_vtb = nxt[4] if nxt else None

    nc.compile()
    return nc


def prep_inputs3(x, masks, Wq, Wk, Wv, mask_proj, proj_w, proj_b):
    f16 = np.float16

    xhatT = np.zeros((B, C, NP), np.float32)
    xhatT[:, :, :N] = x.transpose(0, 2, 1)
    xta_full = np.ascontiguousarray(
        xhatT.reshape(B, KO, 128, NP).transpose(0, 2, 1, 3)).astype(f16)

    def wfold(w, scale=1.0):
        return np.ascontiguousarray(
            (w * scale).reshape(KO, 128, -1).transpose(1, 0, 2)).astype(f16)

    wqp = wfold(Wq, SCALE)
    wkp = wfold(Wk)

    # mask maps: mk[m, k, q] zero-padded
    mkp = np.zeros((ML, NP, NQ), np.float32)
    mkp[:, :N, :N] = masks.transpose(2, 1, 0)

    in_maps = []
    for c in range(8):
        hg, s = c // 2, c % 2
        H0 = NH * hg

        wvh = np.zeros((C, NH * VW), np.float32)
        for hh in range(NH):
            h = H0 + hh
            wvh[:, hh * VW:hh * VW + HD] = Wv[:, h * HD:(h + 1) * HD]
        wvp = np.ascontiguousarray(
            wvh.reshape(KO, 128, -1).transpose(1, 0, 2)).astype(f16)

        pw2p = np.ascontiguousarray(proj_w[H0 * HD:(H0 + 2) * HD]).astype(f16)
        pw1p = np.ascontiguousarray(proj_w[(H0 + 2) * HD:(H0 + 3) * HD]).astype(f16)

        # mw[p, hh, g, j, q] = sum_m mask_proj[m, g*LH+H0+hh] * mk[m, j*128+p, q]
        mp_sub = np.empty((ML, NH, GH), np.float32)
        for hh in range(NH):
            for g in range(GH):
                mp_sub[:, hh, g] = mask_proj[:, g * LH + H0 + hh]
        mwf = np.einsum("mkq,mhg->hgkq", mkp, mp_sub)       # [NH, GH, NP, NQ]
        mwp = np.ascontiguousarray(
            mwf.reshape(NH, GH, NJ, 128, NQ).transpose(3, 0, 1, 2, 4)).astype(f16)

        in_maps.append({
            "xta": xta_full[s * BC:(s + 1) * BC],
            "wq": wqp, "wk": wkp, "wv": wvp,
            "pw2": pw2p, "pw1": pw1p, "mw": mwp,
        })
    return in_maps


_NC3 = None


def get_nc3():
    global _NC3
    if _NC3 is None:
        _NC3 = build_nc3()
    return _NC3


def kernel3(x, masks, Wq, Wk, Wv, mask_proj, proj_w, proj_b):
    x = np.asarray(x, np.float32)
    in_maps = prep_inputs3(
        x, np.asarray(masks, np.float32), np.asarray(Wq, np.float32),
        np.asarray(Wk, np.float32), np.asarray(Wv, np.float32),
        np.asarray(mask_proj, np.float32), np.asarray(proj_w, np.float32),
        np.asarray(proj_b, np.float32))
    res = bass_utils.run_bass_kernel_spmd(get_nc3(), in_maps, core_ids=list(range(8)))
    out = np.zeros((B, N, C), np.float32)
    for c in range(8):
        hg, s = c // 2, c % 2
        out[s * BC:(s + 1) * BC] += np.asarray(res.results[c]["op"], np.float32)[:, :N, :]
    return (out + np.asarray(proj_b, np.float32)).astype(np.float32)


def kernel(x, masks, Wq, Wk, Wv, mask_proj, proj_w, proj_b):
    return kernel3(x, masks, Wq, Wk, Wv, mask_proj, proj_w, proj_b)


# revision 4
# speedup vs baseline: 1.0124x; 1.0124x over previous
# Trainium2 Bass kernel v3 for nn_Attention_48052094107920 (sparse_attention).
#
# Sharding: core c = (head-group c//2, batch-half c%2). Each core processes
# 4 batches x 3 local heads x full query range. The mask-mix maps
# mw[g,h] = sum_m mask_proj[m, g*12+h] * masks_m are folded on the host
# (batch-independent weight fusion) and enter as fp16 inputs.
#
# On-chip layout: "k-major" score tiles S^T[k, q] (k on partitions); x^T is
# zero-padded to [768, 640] (no ones-row; the softmax-denominator column of
# v-hat is memset to 1 on chip). exp uses a -30 bias on padded k rows.
# Softmax: Z (ones-column of p@v PSUM) is copied to SBUF with the o rows,
# broadcast across partitions by DMA, and o/Z runs as an f32 divide (avoids
# fp16-subnormal 1/Z). Out-projection partials stage through fp16 SBUF.
#
# PSUM (8 banks): ppA [128,512]x2 (projections + out-proj), psS [128,4,256]x2
# (score rows j0-3), psT [128,256]x1 (score row j4 / pv tail), pov [65,512]x1.
#
# Engine split: PE matmuls; DVE most mask-mix + divides; Pool (gpsimd) rest of
# mask-mix + most psum->sbuf copies + memsets; ACT exp + wide score copies.

import numpy as np

import concourse.bass as bass
import concourse.bacc as bacc_mod
import concourse.mybir as mybir
import concourse.tile as tile
from concourse import bass_utils

BF = mybir.dt.float16
F32 = mybir.dt.float32
AF = mybir.ActivationFunctionType
OP = mybir.AluOpType

B, N, C = 8, 577, 768
GH, LH, ML, HD = 3, 12, 3, 64
NH = 3            # local heads per core
BC = 4            # batches per core
SCALE = HD ** -0.5
NP, NJ = 640, 5   # padded k tokens, k chunks of 128
NQ = 584          # padded q tokens
KO = 6            # contraction chunks (768 = 6*128)
VW = HD + 1       # 65 cols per head in v-hat: [v | ones]
EXP_NEG = -30.0
QCH = ((0, 256), (256, 512), (512, NQ))        # q chunks (scores / pv)
OQCH = ((0, 128), (128, 256), (256, 384), (384, 512), (512, NQ))  # out rows


def build_nc3(linearize=False):
    nc = bacc_mod.Bacc("TRN2", target_bir_lowering=False, debug=False, num_devices=8)

    xta = nc.dram_tensor("xta", [BC, 128, KO, NP], BF, kind="ExternalInput")
    wq = nc.dram_tensor("wq", [128, KO, GH * HD], BF, kind="ExternalInput")
    wk = nc.dram_tensor("wk", [128, KO, GH * HD], BF, kind="ExternalInput")
    wv = nc.dram_tensor("wv", [128, KO, NH * VW], BF, kind="ExternalInput")
    pw2 = nc.dram_tensor("pw2", [128, C], BF, kind="ExternalInput")
    pw1 = nc.dram_tensor("pw1", [64, C], BF, kind="ExternalInput")
    mw = nc.dram_tensor("mw", [128, NH, GH, NJ, NQ], BF, kind="ExternalInput")
    out = nc.dram_tensor("op", [BC, NQ, C], BF, kind="ExternalOutput")

    with tile.TileContext(nc, linearize=linearize) as tc, \
         tc.tile_pool(name="const", bufs=1) as cpool, \
         tc.tile_pool(name="xb", bufs=2) as xpool, \
         tc.tile_pool(name="work", bufs=2) as wpool, \
         tc.tile_pool(name="amix", bufs=2) as mpool, \
         tc.tile_pool(name="attn", bufs=3) as apool, \
         tc.tile_pool(name="onp", bufs=2) as opool, \
         tc.tile_pool(name="zrp", bufs=2) as zpool, \
         tc.tile_pool(name="psA", bufs=2, space="PSUM") as ppA, \
         tc.tile_pool(name="psS", bufs=2, space="PSUM") as ppS, \
         tc.tile_pool(name="psT", bufs=1, space="PSUM") as ppT, \
         tc.tile_pool(name="psV", bufs=1, space="PSUM") as ppV:

        # x[0] ahead of every other transfer so batch 0 starts ASAP
        xb0 = xpool.tile([128, KO, NP], BF, tag="xb")
        nc.sync.dma_start(xb0[:], xta.ap()[0])

        wq_s = cpool.tile([128, KO, GH * HD], BF, tag="wq")
        nc.sync.dma_start(wq_s[:], wq.ap())
        wk_s = cpool.tile([128, KO, GH * HD], BF, tag="wk")
        nc.sync.dma_start(wk_s[:], wk.ap())
        wv_s = cpool.tile([128, KO, NH * VW], BF, tag="wv")
        nc.sync.dma_start(wv_s[:], wv.ap())
        pw2_s = cpool.tile([128, C], BF, tag="pw2")
        nc.sync.dma_start(pw2_s[:], pw2.ap())
        pw1_s = cpool.tile([64, C], BF, tag="pw1")
        nc.sync.dma_start(pw1_s[:], pw1.ap())
        mw_s = cpool.tile([128, NH, GH, NJ, NQ], BF, tag="mw")

        def load_mw(hh):
            # per-(head, g) chunks, emitted late so xb transfers slip between
            for g in range(GH):
                nc.sync.dma_start(mw_s[:, hh, g], mw.ap()[:, hh, g])

        def phase_a(b, xb=None):
            if xb is None:
                xb = xpool.tile([128, KO, NP], BF, tag="xb")
                nc.sync.dma_start(xb[:], xta.ap()[b])

            q01 = wpool.tile([128, NQ], BF, tag="q01")
            q2 = wpool.tile([64, NQ], BF, tag="q2")
            k01 = wpool.tile([128, NP], BF, tag="k01")
            k2 = wpool.tile([64, NP], BF, tag="k2")
            vtb = wpool.tile([128, NJ, NH * VW], BF, tag="vtb")

            # q projection: [192 rows, NQ]; (512, 72) free chunks
            for msl, mp, dst in ((slice(0, 128), 128, q01), (slice(128, 192), 64, q2)):
                for n0, n1 in ((0, 512), (512, NQ)):
                    ps = ppA.tile([128, 512], F32, tag="bigA", name="psA")[:mp, : n1 - n0]
                    for o in range(KO):
                        nc.tensor.matmul(ps, wq_s[:, o, msl], xb[:, o, n0:n1],
                                         start=(o == 0), stop=(o == KO - 1))
                    nc.scalar.copy(dst[:mp, n0:n1], ps)

            # k projection: [192 rows, NP]; (512, 128) free chunks
            for msl, mp, dst in ((slice(0, 128), 128, k01), (slice(128, 192), 64, k2)):
                for n0, n1 in ((0, 512), (512, NP)):
                    ps = ppA.tile([128, 512], F32, tag="bigA", name="psA")[:mp, : n1 - n0]
                    for o in range(KO):
                        nc.tensor.matmul(ps, wk_s[:, o, msl], xb[:, o, n0:n1],
                                         start=(o == 0), stop=(o == KO - 1))
                    nc.vector.tensor_copy(dst[:mp, n0:n1], ps)

            # v-hat projection: rows k, cols [v_h | 0] per head; ones col memset
            for kc in range(NJ):
                ps = ppA.tile([128, 512], F32, tag="bigA", name="psA")[:, : NH * VW]
                for o in range(KO):
                    nc.tensor.matmul(ps, xb[:, o, kc * 128:(kc + 1) * 128], wv_s[:, o, :],
                                     start=(o == 0), stop=(o == KO - 1))
                nc.scalar.copy(vtb[:, kc, :], ps)
            # ones column only on VALID k rows: padded k (j=4, p>=65) stays 0,
            # so pad keys contribute nothing to o or Z and exp needs no bias
            nc.gpsimd.memset(vtb[:, 0:4, HD::VW], 1.0)
            nc.gpsimd.memset(vtb[0:65, 4, HD::VW], 1.0)
            return q01, q2, k01, k2, vtb

        def phase_b_scores(b, q01, q2, k01, k2, vtb):
            def qg(g):
                return (q01[0:64], q01[64:128], q2[0:64])[g]

            def kg(g):
                return (k01[0:64], k01[64:128], k2[0:64])[g]

            # scores S^T[k, q] per global head -> fp16 ssb
            ssb = wpool.tile([128, NJ, GH, NQ], BF, tag="ssb")
            for g in range(GH):
                for qi, (q0, q1) in enumerate(QCH):
                    qw = q1 - q0
                    ps = ppS.tile([128, 4, 256], F32, tag="sc", name="psS")
                    pt = ppT.tile([128, 256], F32, tag="st", name="psT")
                    for j in range(4):
                        nc.tensor.matmul(ps[:, j, :qw], kg(g)[:, j * 128:(j + 1) * 128],
                                         qg(g)[:, q0:q1], start=True, stop=True)
                    nc.tensor.matmul(pt[:, :qw], kg(g)[:, 512:640],
                                     qg(g)[:, q0:q1], start=True, stop=True)
                    nc.scalar.copy(ssb[:, 0:4, g, q0:q1], ps[:, :, :qw])
                    nc.vector.tensor_copy(ssb[:, 4, g, q0:q1], pt[:, :qw])
            return ssb

        def phase_b_rest(b, ssb, vtb):
            on2 = opool.tile([128, NQ], BF, tag="on2")
            on3 = opool.tile([64, NQ], BF, tag="on3")
            ont = opool.tile([64, NQ], BF, tag="ont")

            for hh in range(NH):
                # mask-mix: at = sum_g ssb_g * mw[hh, g]   (DVE/Pool split,
                # j-halved so exp and p@v can start before the full chain ends)
                at = mpool.tile([128, NJ, NQ], BF, tag="at")
                tt = mpool.tile([128, NJ, NQ], BF, tag="tt")
                t2 = mpool.tile([128, NJ, NQ], BF, tag="t2")
                e = apool.tile([128, NJ, NQ], BF, tag="e")
                for hi, jh in enumerate((slice(0, 3), slice(3, NJ))):
                    tmul = nc.vector.tensor_mul if hi == 0 else nc.gpsimd.tensor_mul
                    nc.vector.tensor_mul(at[:, jh], ssb[:, jh, 0], mw_s[:, hh, 0, jh])
                    nc.gpsimd.tensor_mul(tt[:, jh], ssb[:, jh, 1], mw_s[:, hh, 1, jh])
                    tmul(t2[:, jh], ssb[:, jh, 2], mw_s[:, hh, 2, jh])
                    nc.vector.tensor_add(at[:, jh], at[:, jh], tt[:, jh])
                    tadd = nc.gpsimd.tensor_add if hi == 0 else nc.vector.tensor_add
                    tadd(at[:, jh], at[:, jh], t2[:, jh])
                    nc.scalar.activation(e[:, jh], at[:, jh], AF.Exp)

                # p@v; Z lands in row 64 (early-j MMs overlap the j3:5 exp)
                pov = ppV.tile([VW, 512], F32, tag="ov", name="psV")
                pvt = ppT.tile([128, 256], F32, tag="st", name="psT")
                for qi, (q0, q1) in enumerate(QCH):
                    dstp = pov[:, q0:q1] if qi < 2 else pvt[:VW, : q1 - q0]
                    for j in range(NJ):
                        nc.tensor.matmul(dstp, vtb[:, j, hh * VW:(hh + 1) * VW],
                                         e[:, j, q0:q1], start=(j == 0), stop=(j == NJ - 1))
                zrc = zpool.tile([1, NQ], F32, tag="zrc")
                nc.vector.reciprocal(zrc[0:1, 0:512], pov[64:65, :])
                nc.vector.reciprocal(zrc[0:1, 512:NQ], pvt[64:65, : NQ - 512])
                zrt = zpool.tile([64, NQ], F32, tag="zr")
                nc.gpsimd.partition_broadcast(zrt[:], zrc[0:1, :])
                dst = (on2[0:64], ont[0:64], on3[0:64])[hh]
                nc.vector.tensor_mul(dst[:, 0:512], pov[0:64], zrt[:, 0:512])
                nc.vector.tensor_mul(dst[:, 512:NQ], pvt[0:64, : NQ - 512],
                                     zrt[:, 512:NQ])
                if hh == 1:
                    nc.sync.dma_start(on2[64:128, :], ont[:, :])

            # output projection partials -> fp16 staging -> DRAM
            for q0, q1 in OQCH:
                ow = q1 - q0
                outsb = opool.tile([128, C], BF, tag="outsb")
                for i, (n0, n1) in enumerate(((0, 512), (512, C))):
                    ps = ppA.tile([128, 512], F32, tag="bigA", name="psA")[:ow, : n1 - n0]
                    nc.tensor.matmul(ps, on2[:, q0:q1], pw2_s[:, n0:n1],
                                     start=True, stop=False)
                    nc.tensor.matmul(ps, on3[:, q0:q1], pw1_s[:, n0:n1],
                                     start=False, stop=True)
                    nc.scalar.copy(outsb[:ow, n0:n1], ps)
                nc.sync.dma_start(out.ap()[b, q0:q1, :], outsb[:ow, :])

        # software pipeline: batch b's scores go to the PE first, then batch
        # b+1's projections (they fill the PE while b's mix/exp/pv chain runs
        # on DVE/Pool/ACT), then the rest of batch b. mw chunks are emitted
        # late so the xb transfers stay ahead of them in the SP queue.
        prev = phase_a(0, xb=xb0)
        cur_s = phase_b_scores(0, *prev)
        cur_vtb = prev[4]
        for b in range(BC):
            nxt = phase_a(b + 1) if b + 1 < BC else None
            if b == 0:
                for hh in range(NH):
                    load_mw(hh)
            nxt_s = phase_b_scores(b + 1, *nxt) if nxt else None
            phase_b_rest(b, cur_s, cur_vtb)
            cur_s = nxt_s
            cur_vtb = nxt[4] if nxt else None

    nc.compile()
    return nc


def prep_inputs3(x, masks, Wq, Wk, Wv, mask_proj, proj_w, proj_b):
    f16 = np.float16

    xhatT = np.zeros((B, C, NP), np.float32)
    xhatT[:, :, :N] = x.transpose(0, 2, 1)
    xta_full = np.ascontiguousarray(
        xhatT.reshape(B, KO, 128, NP).transpose(0, 2, 1, 3)).astype(f16)

    def wfold(w, scale=1.0):
        return np.ascontiguousarray(
            (w * scale).reshape(KO, 128, -1).transpose(1, 0, 2)).astype(f16)

    wqp = wfold(Wq, SCALE)
    wkp = wfold(Wk)

    # mask maps: mk[m, k, q] zero-padded
    mkp = np.zeros((ML, NP, NQ), np.float32)
    mkp[:, :N, :N] = masks.transpose(2, 1, 0)

    in_maps = []
    for c in range(8):
        hg, s = c // 2, c % 2
        H0 = NH * hg

        wvh = np.zeros((C, NH * VW), np.float32)
        for hh in range(NH):
            h = H0 + hh
            wvh[:, hh * VW:hh * VW + HD] = Wv[:, h * HD:(h + 1) * HD]
        wvp = np.ascontiguousarray(
            wvh.reshape(KO, 128, -1).transpose(1, 0, 2)).astype(f16)

        pw2p = np.ascontiguousarray(proj_w[H0 * HD:(H0 + 2) * HD]).astype(f16)
        pw1p = np.ascontiguousarray(proj_w[(H0 + 2) * HD:(H0 + 3) * HD]).astype(f16)

        # mw[p, hh, g, j, q] = sum_m mask_proj[m, g*LH+H0+hh] * mk[m, j*128+p, q]
        mp_sub = np.empty((ML, NH, GH), np.float32)
        for hh in range(NH):
            for g in range(GH):
                mp_sub[:, hh, g] = mask_proj[:, g * LH + H0 + hh]
        mwf = np.einsum("mkq,mhg->hgkq", mkp, mp_sub)       # [NH, GH, NP, NQ]
        mwp = np.ascontiguousarray(
            mwf.reshape(NH, GH, NJ, 128, NQ).transpose(3, 0, 1, 2, 4)).astype(f16)

        in_maps.append({
            "xta": xta_full[s * BC:(s + 1) * BC],
            "wq": wqp, "wk": wkp, "wv": wvp,
            "pw2": pw2p, "pw1": pw1p, "mw": mwp,
        })
    return in_maps


_NC3 = None


def get_nc3():
    global _NC3
    if _NC3 is None:
        _NC3 = build_nc3()
    return _NC3


def kernel3(x, masks, Wq, Wk, Wv, mask_proj, proj_w, proj_b):
    x = np.asarray(x, np.float32)
    in_maps = prep_inputs3(
        x, np.asarray(masks, np.float32), np.asarray(Wq, np.float32),
        np.asarray(Wk, np.float32), np.asarray(Wv, np.float32),
        np.asarray(mask_proj, np.float32), np.asarray(proj_w, np.float32),
        np.asarray(proj_b, np.float32))
    res = bass_utils.run_bass_kernel_spmd(get_nc3(), in_maps, core_ids=list(range(8)))
    out = np.zeros((B, N, C), np.float32)
    for c in range(8):
        hg, s = c // 2, c % 2
        out[s * BC:(s + 1) * BC] += np.asarray(res.results[c]["op"], np.float32)[:, :N, :]
    return (out + np.asarray(proj_b, np.float32)).astype(np.float32)


def kernel(x, masks, Wq, Wk, Wv, mask_proj, proj_w, proj_b):
    return kernel3(x, masks, Wq, Wk, Wv, mask_proj, proj_w, proj_b)
